# revision 32
# baseline (speedup 1.0000x reference)
"""Trainium2 Bass kernel for nn_LConvBilin (lattice gauge bilinear conv).

fp16 redesign, 8-core SPMD: V=16384 sites split contiguously across 8 cores
(2048 sites/core, 16 tiles of 128 sites on SBUF partitions).

DVE tensor_tensor runs at 2 elem/cycle in fp16 when every operand's innermost
AP dim is stride-1, even-length, and 4B-aligned; all product layouts here are
engineered for that. Reductions run at 1x regardless, so products are merged
into few large TTs and reduces kept minimal.

Per-tile pipeline:
  P1 (DVE): one TT [c',a,i | m,k | t,j] over supertiles U2D x WT2 (term-pairs
      t = {uR-part, uI-part} with signs folded host-side), one X-reduce over
      (t,j) -> V [c',a,i,m,k].
  V2 (ACT): 4 strided copies arranging V into [c'o,a,m,i,tk] term-pair form.
  P2 (DVE): UP2R = DMA-broadcast of UP2 over (m,i); one TT V2 x UP2R; two
      X-reduces (per c'o) writing the transport block of TALL directly.
  stage C (ACT+PE): gather TALL into [128,121] pair+diag blocks, PE-transpose
      (fp16), 6 matmuls against host-folded weight matrix -> M in PSUM; ACT
      evacuates to MS [c,t,u,v(pad 10),j] fp16.
  stage E (DVE): 8 TTs (bcast w2-variants x MS slices) into EBIG, one XY-reduce
      over (term, v, j) for both c' at once, one TT adding the unit term.

kernel(x, weight) takes FULL inputs, returns FULL output.
"""
import re
import sys

import numpy as np

sys.path.insert(0, "/opt/trn_rl_repo")

DIMS = (16, 16, 8, 8)
V = 16384
N_CORES = 8
S = V // N_CORES
NT = S // 128
PAIRS = [(0, 1), (0, 2), (1, 2)]

_CACHE = {}
SPLIT_WAITS = True


# ---------------------------------------------------------------- tile fixes
def _apply_tile_fixes():
    """This walrus build allows very few semaphore waits per instruction.
    Split the global-clock drain wait across single-wait sync NOPs."""
    if _CACHE.get("fixed"):
        return
    from concourse.tile import TileContext
    from concourse.vector_clock import ScopedClock, VectorClock

    def _clock_values(vc):
        m = re.match(r"VectorClock\(\[(.*)\]\)", repr(vc))
        return [int(x) for x in m.group(1).split(",")]

    def _drain_and_barrier_split(self, tick_clock, wait_clock):
        vals = _clock_values(tick_clock.global_clock)
        for p, val in [(p, v) for p, v in enumerate(vals) if v > 0]:
            v = VectorClock()
            v.require_at_least(p, val)
            nop_inst = self.nc.sync.nop(nofuse=True, hint="drain_split_wait")
            wait_clock.add_sem_waits(nop_inst.ins, ScopedClock({None: v}))
        self.nc.sync.drain()
        self.nc.all_engine_barrier()
        assert self.sems is not None
        popped = self.nc._tile_sem_poison_stack.pop()
        assert popped is self._sem_poison
        self.nc.clear_and_free_semaphores(list(self.sems.allocated().values()))
        self.nc.all_engine_barrier()

    TileContext._drain_and_barrier = _drain_and_barrier_split
    _CACHE["fixed"] = True


def _split_sync_waits(nc, cap=1):
    import concourse.mybir as mybir

    for fn in nc.m.functions:
        for bb in fn.blocks:
            out = []
            for inst in bb.instructions:
                si = inst.sync_info
                if si is not None and si.on_wait and len(si.on_wait) > cap:
                    waits = list(si.on_wait)
                    for i in range(cap, len(waits), cap):
                        nop = mybir.InstNoOp(
                            name=f"{inst.name}-wsplit{i}", ins=[], outs=[]
                        )
                        nop.engine = inst.engine
                        nop.sync_info = mybir.SyncInfo(
                            on_wait=waits[i : i + cap], on_update=[]
                        )
                        nop.bass_nofuse = True
                        out.append(nop)
                    si.on_wait = waits[:cap]
                out.append(inst)
            bb.instructions = out


# ---------------------------------------------------------------- program
def _build_program():
    import concourse.bass as bass
    import concourse.mybir as mybir
    from concourse.masks import make_identity
    from concourse.tile import TileContext

    _apply_tile_fixes()
    F32 = mybir.dt.float32
    F16 = mybir.dt.float16
    MULT = mybir.AluOpType.mult
    ADD = mybir.AluOpType.add

    nc = bass.Bass()
    U2D = nc.dram_tensor("U2D", [S, 144], F16, kind="ExternalInput")
    WT2 = nc.dram_tensor("WT2", [S, 1728], F16, kind="ExternalInput")
    UP2 = nc.dram_tensor("UP2", [S, 1728], F16, kind="ExternalInput")
    XW = nc.dram_tensor("XW", [S, 72], F16, kind="ExternalInput")
    W2S = nc.dram_tensor("W2S", [S, 288], F16, kind="ExternalInput")
    WTS = nc.dram_tensor("WTS", [81, 648], F16, kind="ExternalInput")
    YS = nc.dram_tensor("YS", [S, 72], F16, kind="ExternalOutput")

    def AP(t, off, dims):
        return bass.AP(t.tensor, t.offset + off, [list(t.ap[0])] + dims)

    with TileContext(nc) as tc:
        with (
            nc.allow_low_precision(reason="fp16 kernel, tol 2e-2"),
            tc.tile_pool(name="const", bufs=1) as cpool,
            tc.tile_pool(name="work", bufs=3) as pool,
            tc.tile_pool(name="big", bufs=2) as bigpool,
            tc.tile_pool(name="ps_tr", bufs=2, space="PSUM") as ps_tr,
            tc.tile_pool(name="ps_mm", bufs=2, space="PSUM") as ps_mm,
        ):
            identf = cpool.tile([128, 128], F32)
            make_identity(nc, identf[:, :])
            idf = cpool.tile([128, 128], F16)
            nc.scalar.copy(idf[:, :], identf[:, :])
            wtsb = cpool.tile([81, 648], F16)
            nc.sync.dma_start(wtsb[:, :], WTS[:, :])

            def emit_tile(t):
                rows = slice(t * 128, (t + 1) * 128)
                u2d = pool.tile([128, 144], F16, tag="u2d")
                wt2 = bigpool.tile([128, 1728], F16, tag="wt2")
                up2 = bigpool.tile([128, 1728], F16, tag="up2")
                w2s = pool.tile([128, 288], F16, tag="w2s")
                tall = pool.tile([128, 360], F16, tag="tall")
                nc.sync.dma_start(u2d[:, :], U2D[rows, :])
                nc.sync.dma_start(wt2[:, :], WT2[rows, :])
                nc.sync.dma_start(up2[:, :], UP2[rows, :])
                nc.sync.dma_start(w2s[:, :], W2S[rows, :])
                nc.sync.dma_start(tall[:, 0:72], XW[rows, :])

                # UP2R [c'o,a,(m,i)rep12,l,tk] via SBUF->SBUF broadcast DMA


                # ---- P1: one TT + one reduce ----
                prod1 = bigpool.tile([128, 1728], F16, tag="prod1")
                nc.vector.tensor_tensor(
                    out=AP(prod1, 0, [[72, 24], [6, 12], [1, 6]]),
                    in0=AP(u2d, 0, [[6, 24], [0, 12], [1, 6]]),
                    in1=AP(wt2, 0, [[72, 24], [6, 12], [1, 6]]),
                    op=MULT,
                )
                # P1 reduction as a pair-tree (tj blocks are j-major/t-inner
                # so the first two folds run on even stride-1 pairs at 2x).
                # vv written m-major [c'][a][m][i][k] so the V2 copies can
                # flatten (a,m) into 3-dim ACT APs.
                vv = pool.tile([128, 288], F16, tag="vv")
                nc.vector.tensor_reduce(
                    out=AP(vv, 0, [[36, 8], [3, 3], [9, 4], [1, 3]]),
                    in_=AP(prod1, 0, [[6, 288], [1, 6]]),
                    axis=mybir.AxisListType.X,
                    op=ADD,
                )

                # ---- V2 build [c'o,a,m,i,tk] (ACT, 4 copies) ----
                # vv strides: k1(3) i3(3) m9(4) a36(4) c'144(2)
                # v2 strides: tk1(6) i6(3) m18(4) a72(4) c'o288(2)
                v2 = pool.tile([128, 576], F16, tag="v2")
                for co, tt_, voff in (
                    (0, 0, 0), (0, 1, 144), (1, 0, 144), (1, 1, 0)
                ):
                    nc.scalar.copy(
                        AP(v2, 288 * co + 3 * tt_, [[18, 16], [6, 3], [1, 3]]),
                        AP(vv, voff, [[9, 16], [3, 3], [1, 3]]),
                    )

                # ---- P2: one TT (up2 broadcast over (m,i) via stride-0)
                #      + two reduces into TALL T-part ----
                prod2 = bigpool.tile([128, 1728], F16, tag="prod2")
                nc.vector.tensor_tensor(
                    out=AP(prod2, 0, [[18, 96], [6, 3], [1, 6]]),
                    in0=AP(v2, 0, [[6, 96], [0, 3], [1, 6]]),
                    in1=AP(up2, 0, [[18, 96], [6, 3], [1, 6]]),
                    op=MULT,
                )
                # TALL [ch20][r3][c2][q3]; T-part off = 72 + 72a+18m+6i+3c'+l
                for co in range(2):
                    nc.vector.tensor_reduce(
                        out=AP(tall, 72 + 3 * co, [[18, 16], [6, 3], [1, 3]]),
                        in_=AP(prod2, 864 * co,
                               [[54, 16], [18, 3], [6, 3], [1, 6]]),
                        axis=mybir.AxisListType.X,
                        op=ADD,
                    )

                # ---- gathers -> [128,121] blocks, PE transpose, evac ----
                tqp, tqd = [], []
                for pi, (p_, P_) in enumerate(PAIRS):
                    gq = pool.tile([128, 121], F16, tag=f"gq{pi}")
                    nc.scalar.copy(
                        AP(gq, 0, [[1, 80]]),
                        AP(tall, 6 * p_ + P_,
                           [[3, 2], [5 * (P_ - p_), 2], [18, 20]]),
                    )
                    nc.scalar.copy(
                        AP(gq, 80, [[1, 40]]),
                        AP(tall, 7 * pi, [[3, 2], [18, 20]]),
                    )
                    nc.gpsimd.memset(gq[:, 120:121], 1.0)
                    ptp = ps_tr.tile([128, 128], F16, tag="ptp")
                    nc.tensor.transpose(ptp[0:80, :], gq[:, 0:80], idf[:, :])
                    sp = pool.tile([80, 128], F16, tag=f"tqp{pi}")
                    nc.scalar.copy(sp[:, :], ptp[0:80, :])
                    ptd = ps_tr.tile([64, 128], F16, tag="ptd")
                    nc.tensor.transpose(ptd[0:41, :], gq[:, 80:121], idf[:, :])
                    sd = pool.tile([41, 128], F16, tag=f"tqd{pi}")
                    nc.scalar.copy(sd[:, :], ptd[0:41, :])
                    tqp.append(sp)
                    tqd.append(sd)

                # ---- stage C: 6 matmuls; MS [c2][t3][u4][v10][j3] = 720 ----
                ms = bigpool.tile([128, 720], F16, tag="ms")
                for pi, (p_, P_) in enumerate(PAIRS):
                    mm = ps_mm.tile([128, 144], F32, tag="mmp")
                    nc.tensor.matmul(
                        mm[:, :], tqp[pi][0:80, :],
                        wtsb[0:80, 144 * pi : 144 * pi + 144],
                        start=True, stop=True,
                    )
                    for ordv, (r, q) in enumerate(((p_, P_), (P_, p_))):
                        nc.scalar.copy(
                            AP(ms, 120 * q + r, [[30, 4], [3, 9], [360, 2]]),
                            AP(mm, 72 * ordv, [[1, 72]]),
                        )
                for r in range(3):
                    mm = ps_mm.tile([128, 72], F32, tag="mmd")
                    nc.tensor.matmul(
                        mm[:, :], tqd[r][0:41, :],
                        wtsb[0:41, 432 + 72 * r : 504 + 72 * r],
                        start=True, stop=True,
                    )
                    nc.scalar.copy(
                        AP(ms, 120 * r + r, [[30, 4], [3, 9], [360, 2]]),
                        AP(mm, 0, [[1, 72]]),
                    )

                return rows, w2s, ms

            def emit_back(state):
                # ---- stage E: rebig half on DVE, imbig half on GpSimd ----
                rows, w2s, ms = state
                ebig = bigpool.tile([128, 3456], F16, tag="ebig")
                cms = [0, 1, 0, 1, 1, 0, 1, 0]
                vbs = [0, 0, 4, 4, 0, 0, 4, 4]
                for k_ in range(8):
                    eng = nc.vector if k_ < 4 else nc.gpsimd
                    eng.tensor_tensor(
                        out=AP(ebig, 432 * k_, [[12, 36], [1, 12]]),
                        in0=AP(w2s, 36 * k_, [[0, 12], [12, 3], [1, 12]]),
                        in1=AP(ms, 360 * cms[k_] + 3 * vbs[k_],
                               [[30, 12], [0, 3], [1, 12]]),
                        op=MULT,
                    )
                # imbig: one gpsimd tree-fold, DVE finishes both reductions
                efold = pool.tile([128, 864], F16, tag="efold")
                nc.gpsimd.tensor_tensor(
                    out=AP(efold, 0, [[1, 864]]),
                    in0=AP(ebig, 1728, [[1, 864]]),
                    in1=AP(ebig, 2592, [[1, 864]]),
                    op=ADD,
                )
                out2 = pool.tile([128, 72], F16, tag="out2")
                nc.vector.tensor_reduce(
                    out=AP(out2, 0, [[1, 36]]),
                    in_=AP(ebig, 0, [[12, 36], [432, 4], [1, 12]]),
                    axis=mybir.AxisListType.XY,
                    op=ADD,
                )
                nc.vector.tensor_reduce(
                    out=AP(out2, 36, [[1, 36]]),
                    in_=AP(efold, 0, [[12, 36], [432, 2], [1, 12]]),
                    axis=mybir.AxisListType.XY,
                    op=ADD,
                )
                nc.vector.tensor_tensor(
                    out=AP(out2, 0, [[36, 2], [12, 3], [3, 4], [1, 3]]),
                    in0=AP(out2, 0, [[36, 2], [12, 3], [3, 4], [1, 3]]),
                    in1=AP(ms, 24, [[360, 2], [120, 3], [30, 4], [1, 3]]),
                    op=ADD,
                )
                nc.sync.dma_start(YS[rows, :], out2[:, :])

            pending = []
            for t in range(NT):
                pending.append(emit_tile(t))
                if len(pending) > 1:
                    emit_back(pending.pop(0))
            for st in pending:
                emit_back(st)
    if SPLIT_WAITS:
        _split_sync_waits(nc)
    return nc


# ---------------------------------------------------------------- host prep
def _host_prep(x, weight):
    x = np.ascontiguousarray(x, dtype=np.float32)
    weight = np.ascontiguousarray(weight, dtype=np.float32)
    u = x[0, :, :4]          # [V, a, i, j, c]
    w = x[0, :, 4:]          # [V, m, i, j, c]
    wgrid = w.reshape(DIMS + (4, 3, 3, 2))
    ws = np.stack([np.roll(wgrid, -1, axis=a).reshape(V, 4, 3, 3, 2)
                   for a in range(4)], axis=1)       # [V, a, m, j, k, c]

    # tj blocks j-major / t-inner (pairs) for the P1 tree-reduce
    uu = np.stack([u[..., 0], u[..., 1]], axis=-1)   # [V,a,i,j,t]
    u2d = np.broadcast_to(uu[:, None], (V, 2, 4, 3, 3, 2)).reshape(V, 144)

    wsR, wsI = ws[..., 0], ws[..., 1]                # [V,a,m,j,k]
    re_blk = np.stack([wsR, -wsI], axis=-1)          # [V,a,m,j,k,t]
    im_blk = np.stack([wsI, wsR], axis=-1)
    wt2 = np.stack([re_blk, im_blk], axis=1)         # [V,c',a,m,j,k,t]
    wt2 = wt2.transpose(0, 1, 2, 3, 5, 4, 6)         # [V,c',a,m,k,j,t]
    wt2 = np.broadcast_to(
        wt2[:, :, :, None], (V, 2, 4, 3, 4, 3, 3, 2)
    ).reshape(V, 1728)

    uR, uI = u[..., 0], u[..., 1]                    # [V,a,l,k]
    up2 = np.stack([
        np.stack([uR, uI], axis=-2),
        np.stack([uR, -uI], axis=-2),
    ], axis=1).reshape(V, 2, 4, 18)
    # replicate over (m,i)=12 -> dense in1 so P2-TT keeps 2x mode
    up2 = np.broadcast_to(
        up2[:, :, :, None], (V, 2, 4, 12, 18)).reshape(V, 1728)

    xw = w.transpose(0, 1, 2, 4, 3).reshape(V, 72)   # [V,m,i,c,k]

    wR, wI = w[..., 0], w[..., 1]                    # [V, m, i, j]
    wb = wR.transpose(0, 2, 1, 3)                    # [V,i,v,j] = wR[v,i,j]
    wbI = wI.transpose(0, 2, 1, 3)
    wbd = wR.transpose(0, 3, 1, 2)
    wbdI = wI.transpose(0, 3, 1, 2)
    w2s = np.stack(
        [wb, -wbI, wbd, wbdI, wb, wbI, wbd, -wbdI], axis=1
    ).reshape(V, 288)

    # WTS: baseline wtsd with columns reordered into pair/diag groups
    o1 = weight[:, :, :20]; o2 = weight[:, :, 20:40]; o3 = weight[:, :, 40]
    wtsd = np.zeros((81, 648), np.float32)
    for r in range(3):
        for q in range(3):
            rq = r * 3 + q
            blk = np.zeros((81, 4, 9, 2), np.float32)
            if r == q:
                for c in range(2):
                    sgn = 1.0 if c == 0 else -1.0
                    for ch in range(20):
                        blk[c * 20 + ch, :, :, c] = (
                            o1[:, :, ch] + sgn * o2[:, :, ch])
                blk[40, :, :, 0] = o3
            else:
                p_, P_ = min(r, q), max(r, q)
                my_ord = 0 if (r, q) == (p_, P_) else 1
                for c in range(2):
                    sgn = 1.0 if c == 0 else -1.0
                    for ch in range(20):
                        blk[c * 40 + my_ord * 20 + ch, :, :, c] = o1[:, :, ch]
                        blk[c * 40 + (1 - my_ord) * 20 + ch, :, :, c] = (
                            sgn * o2[:, :, ch])
            wtsd[:, rq * 72:(rq + 1) * 72] = blk.reshape(81, 72)
    order = [1, 3, 2, 6, 5, 7, 0, 4, 8]   # (01,10),(02,20),(12,21),00,11,22
    wts = np.concatenate(
        [wtsd[:, rq * 72:(rq + 1) * 72] for rq in order], axis=1)

    f16 = np.float16
    return (u2d.astype(f16), wt2.astype(f16), up2.astype(f16),
            xw.astype(f16), w2s.astype(f16), wts.astype(f16))


def kernel(x, weight):
    x = np.asarray(x, dtype=np.float32)
    weight = np.asarray(weight, dtype=np.float32)
    from concourse.bass_utils import run_bass_kernel_spmd

    u2d, wt2, up2, xw, w2s, wts = _host_prep(x, weight)

    if "nc" not in _CACHE:
        _CACHE["nc"] = _build_program()
    nc = _CACHE["nc"]

    in_maps = []
    for r in range(N_CORES):
        sl = slice(r * S, (r + 1) * S)
        in_maps.append({
            "U2D": np.ascontiguousarray(u2d[sl]),
            "WT2": np.ascontiguousarray(wt2[sl]),
            "UP2": np.ascontiguousarray(up2[sl]),
            "XW": np.ascontiguousarray(xw[sl]),
            "W2S": np.ascontiguousarray(w2s[sl]),
            "WTS": wts,
        })
    res = run_bass_kernel_spmd(
        nc, in_maps, list(range(N_CORES)), trace=_CACHE.get("trace", False)
    )
    _CACHE["last_result"] = res
    ys = np.concatenate(
        [np.asarray(res.results[r]["YS"]) for r in range(N_CORES)], axis=0
    ).astype(np.float32)
    # ys [V, c'2, t3, u4, i3] -> out_w [V, u, i, k=t, c']
    out_w = ys.reshape(V, 2, 3, 4, 3).transpose(0, 3, 4, 2, 1)
    out = np.concatenate([x[0, :, :4], out_w], axis=1)[None]
    return out.astype(np.float32)
